# revision 4
# baseline (speedup 1.0000x reference)
"""BEiT self-attention Trainium2 kernel (Bass/Tile), data-parallel over batch on 8 cores.

bf16 layout strategy (per core, 8 batches):
  - hidden pre-transposed on host to feature-major xT [768, 1584] bf16 (padded).
  - Q^T, K^T computed head-dim-major [o, m] in bf16; 1/sqrt(64) folded into wq/bq.
    Q gets bias+cast on ACT, K casts on ACT.
  - V computed seq-major per (batch, j-tile) with a ones column per head
    (65-wide head groups) so the probs@V matmul also yields softmax row-sums.
  - Attention per (batch, head): scoresT[j, i] = k^T.T @ q^T streamed over a
    198-wide i window, two heads paired on opposite PE row halves.
    Softmax bias is folded multiplicatively: u = exp(scores) * expb where
    expb = exp(rel_bias) is precomputed on host (bf16).
  - ctx computed directly seq-major: ct[i, 65] = u[j,i].T @ [V|1][j, 65]
    (u tiles as PE weights) - no PE transpose, no PSUM->SBUF copy.
    ct accumulates 6 heads side by side [i, 390]; normalization is one
    reciprocal + one broadcast-multiply per (batch, i-tile, head-group).
  - Work split into 4 groups of 2 batches; group g's attention interleaves
    with group g+1's projection matmuls to keep the PE stream dense.
"""

from collections import deque

import numpy as np
import ml_dtypes

import concourse.bacc as bacc
import concourse.mybir as mybir
from concourse.tile import TileContext
from concourse.bass import broadcast_tensor_aps as bass_broadcast
from concourse.bass_utils import run_bass_kernel_spmd

B, S, D, H, HD = 64, 197, 768, 12, 64
NCORES = 8
BPC = B // NCORES  # batches per core
F32 = mybir.dt.float32
BF16 = mybir.dt.bfloat16
IW = 198  # i window per j-tile half
IW2 = 2 * IW  # 396
KT = D // 128  # 6 contraction tiles
OT = D // 128  # 6 output-feature tiles
JT = [(0, 128), (128, S - 128)]  # j (key) partition tiles
IT = [(0, 128), (128, S - 128)]  # i (query) partition tiles
MG = 2 * S  # group width (2 batches)
QW = MG + 2  # padded stream width (scores windows reach col 395)
XW = BPC * S + 8  # padded xT dram width
AluOp = mybir.AluOpType
ActFn = mybir.ActivationFunctionType


def build_program(bpc=BPC, group_sizes=None):
    if group_sizes is None:
        group_sizes = (2, 2, 2, 1, 1) if bpc == 8 else (bpc,)
    assert sum(group_sizes) == bpc
    MTOT = bpc * S

    nc = bacc.Bacc("TRN2", target_bir_lowering=False, debug=False, num_devices=1)
    xT_d = nc.dram_tensor("xT", [D, XW], BF16, kind="ExternalInput")
    wqT_d = nc.dram_tensor("wqT", [D, D], BF16, kind="ExternalInput")
    wkT_d = nc.dram_tensor("wkT", [D, D], BF16, kind="ExternalInput")
    wvT_d = nc.dram_tensor("wvT", [D, D], BF16, kind="ExternalInput")
    bq_d = nc.dram_tensor("bq2", [128, OT], F32, kind="ExternalInput")
    bv_d = nc.dram_tensor("bvb", [128, D], BF16, kind="ExternalInput")
    eb_d = nc.dram_tensor("expb", [H, 128, IW2], BF16, kind="ExternalInput")
    on_d = nc.dram_tensor("onec", [128, H], BF16, kind="ExternalInput")
    out_d = nc.dram_tensor("out", [MTOT, D], F32, kind="ExternalOutput")

    with TileContext(nc) as tc:
        with (
            tc.tile_pool(name="const", bufs=1) as cp,
            tc.tile_pool(name="grp", bufs=2) as gp,
            tc.tile_pool(name="work", bufs=3) as wp,
            tc.tile_pool(name="ps", bufs=1, space="PSUM") as pp,
        ):
            wq_t = [
                cp.tile([128, D], BF16, name=f"wq{k}", tag=f"wq{k}") for k in range(KT)
            ]
            wk_t = [
                cp.tile([128, D], BF16, name=f"wk{k}", tag=f"wk{k}") for k in range(KT)
            ]
            wv_t = [
                cp.tile([128, D], BF16, name=f"wv{k}", tag=f"wv{k}") for k in range(KT)
            ]
            bqs = cp.tile([128, OT], F32, tag="bqs")
            bvb = cp.tile([128, D], BF16, tag="bvb")
            onec = cp.tile([128, H], BF16, tag="onec")
            eb_t = [
                cp.tile([128, IW2], BF16, name=f"eb{h}", tag=f"eb{h}") for h in range(H)
            ]

            def load_wq():
                for k in range(KT):
                    nc.sync.dma_start(wq_t[k][:], wqT_d[k * 128 : (k + 1) * 128, :])

            def load_weights():
                for k in range(KT):
                    nc.sync.dma_start(wk_t[k][:], wkT_d[k * 128 : (k + 1) * 128, :])
                for k in range(KT):
                    nc.sync.dma_start(wv_t[k][:], wvT_d[k * 128 : (k + 1) * 128, :])
                nc.sync.dma_start(bqs[:], bq_d[:, :])
                nc.sync.dma_start(bvb[:], bv_d[:, :])
                nc.sync.dma_start(onec[:], on_d[:, :])

            def load_bias():
                for h in range(H):
                    nc.sync.dma_start(eb_t[h][:], eb_d[h, :, :])

            def proj_pieces(g, GB, b0):
                """Emission thunks for group g's projections; last item is the
                ('ctx', dict) sentinel carrying the produced tiles."""
                m0 = b0 * S
                gw = GB * S + 2  # used stream width (<= QW tile width)
                ctx = {}

                def piece_load():
                    xt = [
                        gp.tile([128, QW], BF16, name=f"xt{k}", tag=f"xt{k}")
                        for k in range(KT)
                    ]
                    for k in range(KT):
                        nc.sync.dma_start(
                            xt[k][:, :gw], xT_d[k * 128 : (k + 1) * 128, m0 : m0 + gw]
                        )
                    ctx["xt"] = xt
                    ctx["qt"] = [
                        gp.tile([128, QW], BF16, name=f"qt{o}", tag=f"qt{o}")
                        for o in range(OT)
                    ]
                    ctx["kt"] = [
                        gp.tile([128, QW], BF16, name=f"kt{o}", tag=f"kt{o}")
                        for o in range(OT)
                    ]
                    ctx["vt"] = {}

                yield piece_load

                def piece_qt(o):
                    xt, qt = ctx["xt"], ctx["qt"]
                    ps = pp.tile([128, 512], F32, name="pp", tag="mm512", bufs=4)
                    for ki in range(KT):
                        nc.tensor.matmul(
                            ps[:, :gw],
                            wq_t[ki][:, o * 128 : (o + 1) * 128],
                            xt[ki][:, :gw],
                            start=(ki == 0),
                            stop=(ki == KT - 1),
                        )
                    nc.scalar.activation(
                        qt[o][:, :gw],
                        ps[:, :gw],
                        ActFn.Identity,
                        bias=bqs[:, o : o + 1],
                    )

                def piece_kt(o):
                    xt, kt = ctx["xt"], ctx["kt"]
                    ps = pp.tile([128, 512], F32, name="pp", tag="mm512", bufs=4)
                    for ki in range(KT):
                        nc.tensor.matmul(
                            ps[:, :gw],
                            wk_t[ki][:, o * 128 : (o + 1) * 128],
                            xt[ki][:, :gw],
                            start=(ki == 0),
                            stop=(ki == KT - 1),
                        )
                    nc.scalar.activation(kt[o][:, :gw], ps[:, :gw], ActFn.Identity)

                def piece_v(b, jt):
                    xt = ctx["xt"]
                    j0, jw = JT[jt]
                    v = gp.tile(
                        [128, H * 65], BF16, name=f"v{b}_{jt}", tag=f"v{b}_{jt}"
                    )
                    v3 = v[:jw, :].rearrange("p (h c) -> p h c", c=65)
                    for c0, cw, h0 in [(0, 512, 0), (512, 256, 8)]:
                        nh = cw // 64
                        ps = pp.tile([128, 512], F32, name="pp", tag="mm512", bufs=4)
                        for ki in range(KT):
                            nc.tensor.matmul(
                                ps[:jw, :cw],
                                xt[ki][:, b * S + j0 : b * S + j0 + jw],
                                wv_t[ki][:, c0 : c0 + cw],
                                start=(ki == 0),
                                stop=(ki == KT - 1),
                            )
                        dst = v3[:, h0 : h0 + nh, 0:64]
                        src = ps[:jw, :cw].rearrange("p (h c) -> p h c", c=64)
                        bsl = bvb[:jw, c0 : c0 + cw].rearrange("p (h c) -> p h c", c=64)
                        nc.vector.tensor_tensor(dst, src, bsl, AluOp.add)
                    nc.vector.tensor_copy(
                        v3[:, :, 64:65],
                        onec[:jw, :].rearrange("p (h c) -> p h c", c=1),
                    )
                    ctx["vt"][b, jt] = v

                for o in range(OT):
                    yield (lambda o=o: piece_qt(o))
                for o in range(OT):
                    yield (lambda o=o: piece_kt(o))
                for b in range(GB):
                    for jt in range(2):
                        yield (lambda b=b, jt=jt: piece_v(b, jt))
                yield ("ctx", ctx)

            def att_pieces(GB, b0, ctx):
                """Emission thunks for a group's attention (lagged ctx stage).
                ctx is read lazily: tiles may be created mid-stream."""
                hgstate = {}

                def stage_scores_pair(b, hp):
                    qt, kt = ctx["qt"], ctx["kt"]
                    o = hp // 2
                    sts = [
                        pp.tile([128, 512], F32, name="st", tag="mm512", bufs=4)
                        for _ in range(2)
                    ]
                    # interleave the two heads so consecutive matmuls hit
                    # opposite PE row groups (partitions 0-63 vs 64-127) and
                    # overlap in the array
                    for jt, (j0, jw) in enumerate(JT):
                        for dh in (0, 1):
                            po = dh * 64
                            nc.tensor.matmul(
                                sts[dh][:jw, jt * IW : (jt + 1) * IW],
                                kt[o][po : po + 64, b * S + j0 : b * S + j0 + jw],
                                qt[o][po : po + 64, b * S : b * S + IW],
                                start=True,
                                stop=True,
                            )
                    out = []
                    for dh in (0, 1):
                        e = wp.tile([128, IW2], BF16, name="ee", tag="ee", bufs=4)
                        nc.scalar.activation(e[:, :], sts[dh][:, :IW2], ActFn.Exp)
                        u = wp.tile([128, IW2], BF16, name="uu", tag="uu", bufs=6)
                        nc.vector.tensor_tensor(
                            u[:, :], e[:, :], eb_t[hp + dh][:, :], AluOp.mult
                        )
                        out.append(u)
                    return out

                def stage_ctx(b, h, u, row0):
                    vt = ctx["vt"]
                    hg, hl = h // 6, h % 6
                    if hl == 0:
                        hgstate[b, hg] = [
                            pp.tile([128, 390], F32, name="ct", tag="ct", bufs=4)
                            for _ in range(2)
                        ]
                    cts = hgstate[b, hg]
                    for it, (i0, iw) in enumerate(IT):
                        for jt, (j0, jw) in enumerate(JT):
                            nc.tensor.matmul(
                                cts[it][:iw, hl * 65 : (hl + 1) * 65],
                                u[:jw, jt * IW + i0 : jt * IW + i0 + iw],
                                vt[b, jt][:jw, h * 65 : (h + 1) * 65],
                                start=(jt == 0),
                                stop=(jt == 1),
                            )
                    if hl == 5:
                        for it, (i0, iw) in enumerate(IT):
                            ct3 = cts[it][:iw, :].rearrange("p (h c) -> p h c", c=65)
                            rt = wp.tile([128, 6], F32, name="rt", tag="rt")
                            rt3 = rt[:iw, :].rearrange("p (h c) -> p h c", c=1)
                            nc.vector.reciprocal(rt3, ct3[:, :, 64:65])
                            num = ct3[:, :, 0:64]
                            _, rb3 = bass_broadcast(num, rt3)
                            ob = wp.tile([128, 384], F32, name="ob", tag="ob")
                            nc.vector.tensor_tensor(
                                ob[:iw, :].rearrange("p (h c) -> p h c", c=64),
                                num,
                                rb3,
                                AluOp.mult,
                            )
                            nc.sync.dma_start(
                                out_d[
                                    row0 + i0 : row0 + i0 + iw,
                                    hg * 384 : (hg + 1) * 384,
                                ],
                                ob[:iw, :],
                            )

                pend = deque()
                for b in range(GB):
                    for hp in range(0, H, 2):

                        def piece(b=b, hp=hp):
                            us01 = stage_scores_pair(b, hp)
                            for dh in (0, 1):
                                pend.append((b, hp + dh, us01[dh], (b0 + b) * S))
                            while len(pend) > 4:
                                stage_ctx(*pend.popleft())

                        yield piece

                def flush():
                    while pend:
                        stage_ctx(*pend.popleft())

                yield flush

            def run_proj(gen):
                pieces = []
                ctx = None
                for item in gen:
                    if isinstance(item, tuple) and item[0] == "ctx":
                        ctx = item[1]
                    else:
                        pieces.append(item)
                return pieces, ctx

            b0s = []
            acc = 0
            for GB in group_sizes:
                b0s.append(acc)
                acc += GB

            g0_pieces, prev_ctx = run_proj(proj_pieces(0, group_sizes[0], b0s[0]))
            load_wq()
            g0_pieces[0]()  # xT DMAs right behind the wq tiles
            load_weights()
            for p in g0_pieces[1:]:
                p()
            load_bias()

            def interleave(astream, pstream):
                ratio = max(1, len(astream) // max(1, len(pstream)))
                out = []
                ai = pi = 0
                while ai < len(astream) or pi < len(pstream):
                    for _ in range(ratio):
                        if ai < len(astream):
                            out.append(astream[ai])
                            ai += 1
                    if pi < len(pstream):
                        out.append(pstream[pi])
                        pi += 1
                return out

            ng = len(group_sizes)
            for g in range(1, ng - 1):
                pieces, g_ctx = run_proj(proj_pieces(g, group_sizes[g], b0s[g]))
                for p in interleave(
                    list(att_pieces(group_sizes[g - 1], b0s[g - 1], prev_ctx)), pieces
                ):
                    p()
                prev_ctx = g_ctx

            if ng == 1:
                for p in att_pieces(group_sizes[0], b0s[0], prev_ctx):
                    p()
            else:
                # final window: att(gl-1) interleaved with the last group's
                # load/qt/kt0-2 pieces; kt3-5 + V pieces are deferred into the
                # last group's own attention stream as just-in-time PE filler.
                gl = ng - 1
                pieces, gl_ctx = run_proj(proj_pieces(gl, group_sizes[gl], b0s[gl]))
                pload = pieces[0]
                pqt = pieces[1 : 1 + OT]
                pkt = pieces[1 + OT : 1 + 2 * OT]
                pv = deque(pieces[1 + 2 * OT :])
                window = [pload] + pqt + pkt[:3]
                for p in interleave(
                    list(att_pieces(group_sizes[gl - 1], b0s[gl - 1], prev_ctx)),
                    window,
                ):
                    p()
                apieces = list(att_pieces(group_sizes[gl], b0s[gl], gl_ctx))
                aflush = apieces[-1]
                A = apieces[:-1]
                out_stream = []
                for idx, a in enumerate(A):
                    if 3 <= idx < OT:
                        out_stream.append(pkt[idx])  # kt[idx] just before its pair
                    out_stream.append(a)
                    if pv and idx in (0, 1, 5, 7):
                        out_stream.append(pv.popleft())
                while pv:
                    out_stream.append(pv.popleft())
                out_stream.append(aflush)
                for p in out_stream:
                    p()

    nc.compile()
    return nc


def prep_host_inputs(inputs, bpc=BPC, cores=NCORES):
    """Shared (per-core-identical) tensors + per-core xT shards."""
    hs = np.ascontiguousarray(np.asarray(inputs["hidden_states"], dtype=np.float32))
    wq = np.asarray(inputs["wq"], np.float32)
    bq = np.asarray(inputs["bq"], np.float32)
    wk = np.asarray(inputs["wk"], np.float32)
    wv = np.asarray(inputs["wv"], np.float32)
    bv = np.asarray(inputs["bv"], np.float32)
    bias_table = np.asarray(inputs["bias_table"], np.float32)
    rel_index = np.asarray(inputs["rel_index"])

    bf = ml_dtypes.bfloat16
    scale = np.float32(1.0 / np.sqrt(HD))
    common = {
        "wqT": np.ascontiguousarray(wq.T * scale).astype(bf),
        "wkT": np.ascontiguousarray(wk.T).astype(bf),
        "wvT": np.ascontiguousarray(wv.T).astype(bf),
        "bq2": np.ascontiguousarray((bq * scale).reshape(OT, 128).T),
        "bvb": np.ascontiguousarray(np.broadcast_to(bv, (128, D))).astype(bf),
        "onec": np.ones((128, H), bf),
    }
    rb = bias_table[rel_index]  # [i, j, H]
    bjiT = rb.transpose(2, 1, 0)  # [h, j, i]
    eb = np.zeros((H, 128, IW2), np.float32)
    for jt, (j0, jw) in enumerate(JT):
        eb[:, :jw, jt * IW : jt * IW + S] = np.exp(bjiT[:, j0 : j0 + jw, :])
    common["expb"] = eb.astype(bf)

    in_maps = []
    for c in range(cores):
        xc = hs[c * bpc : (c + 1) * bpc].reshape(bpc * S, D)
        xT = np.zeros((D, XW), np.float32)
        xT[:, : bpc * S] = xc.T
        in_maps.append({"xT": xT.astype(bf), **common})
    return in_maps


_prog_cache = {}


def get_program(bpc=BPC, group_sizes=None):
    key = (bpc, group_sizes)
    if key not in _prog_cache:
        _prog_cache[key] = build_program(bpc, group_sizes)
    return _prog_cache[key]


def kernel(**inputs):
    nc = get_program()
    in_maps = prep_host_inputs(inputs)
    res = run_bass_kernel_spmd(nc, in_maps, list(range(NCORES)))
    out = np.concatenate([res.results[c]["out"] for c in range(NCORES)], axis=0)
    return out.reshape(B, S, D)
